# revision 24
# baseline (speedup 1.0000x reference)
"""Trainium2 Bass kernel for ByteLatentEncoder topk_mean_pooling (segment top-4 mean).

Problem: h [8, 4096, 512] f32, patch_ids [8, 4096] int64 (sorted per row,
values in [0, 1024)).  Output [8, 1024, 512]: per (batch, patch, channel),
mean of the top-min(4, count) *distinct* segment values with the reference's
knockout semantics (ties collapse; exhausted ranks contribute exactly -1e9).

v2 design (one NeuronCore per batch row):
  - Patches are grouped by EXACT count c into device classes c=2..8.  Each
    class gathers its segments as contiguous c-row windows from an fp16 copy
    of h with ONE dma_gather (SWDGE ISA ucode) per class -- no masks, no
    per-token column DMAs, half the bytes of fp32.
  - Tie-free top-4 means are order statistics, so they are computed with
    fp16 partial-sort (bitonic) networks on stock tensor_tensor ops, which
    run at 2 elem/cycle on the DVE (fp32 exactness is only needed for the
    reference's tie-knockout cases, which are routed to the host path).
      c=2..4: out = (sum of all c)/c          (plain adds)
      c=5:    out = (sum5 - min5)/4           (adds + min tree)
      c=6:    top4 = bitonic split of sort4(asc)++sort2(desc) padded
      c=7,8:  one shared W=8 block: sort4(asc) ++ sort4(desc), H=max split;
              c=7 windows read one foreign row that is overwritten by a
              -FLT16_MAX memset plane before the network runs.
  - Host precomputes (exact fp32 reference replica) the rare rows the fp16
    path can't represent: c=1 (copy), c>=9 (top-4 of a wide segment), and
    any patch with an exact in-segment duplicate (knockout -1e9 semantics).
    c=0 rows are zero -- covered by the output zero-init.
  - Results are scaled+cast to fp32 on the Scalar engine (1/min(4,c) per
    class) and written with one dma_scatter_add per class onto the
    zero-initialized output (add-to-zero == copy; pad slots land in a
    sacrificial 1025th row).
"""

import math
from contextlib import ExitStack

import numpy as np

import concourse.bacc as bacc
import concourse.bass as bass
import concourse.mybir as mybir
import concourse.tile as tile
from concourse.bass_utils import run_bass_kernel_spmd
from concourse.library_config import mlp

P = 128
SEQ = 4096
DIM = 512
NPATCH = 1024
K = 4
NEG = -1.0e9
NEGF16 = -65504.0
HB_ROWS = SEQ + 8  # 8 pad rows so full-8 windows of the last patch stay in range
DUMMY = SEQ  # dummy gather row (zeros pad region)
TRASH = NPATCH  # sacrificial scatter row

DEV_CLASSES = (2, 3, 4, 5, 6, 7, 8)


# ---------------------------------------------------------------- host side

def _reference_rows(h_row, starts, counts, pids):
    """Exact fp32 replica of reference() for the given patch ids."""
    out = np.zeros((len(pids), DIM), np.float32)
    for i, p in enumerate(pids):
        c = int(counts[p])
        if c == 0:
            continue
        seg = h_row[starts[p]:starts[p] + c].astype(np.float32)
        work = seg.copy()
        acc = np.zeros(DIM, np.float32)
        n = min(K, c)
        for r in range(n):
            cm = work.max(axis=0)
            acc += cm
            work = np.where(work == cm[None, :], np.float32(NEG), work)
        out[i] = acc / np.float32(n)
    return out


def _find_tie_patches(h_row, starts, counts, cand):
    """Among candidate patch ids (2<=c<=8), those with an exact per-channel
    duplicate anywhere in the segment (conservative superset of the patches
    where reference knockout != plain top-4)."""
    ties = []
    for c in range(2, 9):
        sel = cand[counts[cand] == c]
        if len(sel) == 0:
            continue
        idx = starts[sel, None] + np.arange(c)[None, :]
        seg = h_row[idx]  # [n, c, DIM]
        s = np.sort(seg, axis=1)
        dup = (s[:, 1:, :] == s[:, :-1, :]).any(axis=(1, 2))
        ties.extend(sel[dup].tolist())
    return ties


def build_row(h_row, pid_row):
    starts = np.searchsorted(pid_row, np.arange(NPATCH + 1)).astype(np.int64)
    counts = np.diff(starts)
    starts = starts[:-1]
    cand = np.where((counts >= 2) & (counts <= 8))[0]
    ties = set(_find_tie_patches(h_row, starts, counts, cand))
    cls = {c: [] for c in DEV_CLASSES}
    ovr = []
    for p in range(NPATCH):
        c = int(counts[p])
        if c == 0:
            continue
        if c == 1 or c >= 9 or p in ties:
            ovr.append(p)
        else:
            cls[c].append(p)
    return dict(starts=starts, counts=counts, cls=cls, ovr=ovr)


def wrap16(idx, n_slots):
    """SWDGE idx layout: slot j at [j%16, j//16], replicated to 8 stripes."""
    cols = (n_slots + 15) // 16
    t = np.zeros((16, cols), np.int16)
    for j, v in enumerate(idx):
        t[j % 16, j // 16] = v
    return np.tile(t, (8, 1))


def prepare(h, patch_ids):
    h = np.ascontiguousarray(np.asarray(h, np.float32))
    pid = np.asarray(patch_ids)
    nb = h.shape[0]
    rows = [build_row(h[b], pid[b]) for b in range(nb)]

    # global (compile-time) sizes
    ncls = {c: max(len(r["cls"][c]) for r in rows) for c in DEV_CLASSES}
    Q = {c: max(1, math.ceil(ncls[c] / P)) for c in (2, 3, 4, 5, 6)}
    n7max, n8max = ncls[7], ncls[8]
    assert n7max <= P and n8max <= P, (n7max, n8max)
    assert n7max + n8max <= P, "c7+c8 exceed one q-block; add Q78 support"
    Q[78] = 1
    novr = max(1, max(len(r["ovr"]) for r in rows))
    QO = math.ceil(novr / P)
    sizes = dict(Q=Q, n7max=n7max, n8max=n8max, QO=QO)

    in_maps = []
    for b, r in enumerate(rows):
        st, cn = r["starts"], r["counts"]
        hb = np.concatenate(
            [h[b], np.zeros((HB_ROWS - SEQ, DIM), np.float32)], 0
        ).astype(np.float16)

    # staging row map: class regions (order 78,6,5,4,3,2), override region,
    # one zero row; rix[p] = staging row whose content is out[p]
    aq = {}
    q0 = 0
    for cid in (78, 6, 5, 4, 3, 2):
        aq[cid] = q0
        q0 += Q[cid]
    SQ = q0
    ovr_base = SQ * P
    zero_row = ovr_base + QO * P
    stg_rows = zero_row + 1
    sizes.update(ncls=ncls, SQ=SQ, stg_rows=stg_rows)

    for b, r in enumerate(rows):
        st, cn = rows[b]["starts"], rows[b]["counts"]
        # gather idx tables: reals first, per-row shortfall [n_c, ncmax)
        # reads the dummy row, static tail [ncmax, 128*Q) is -1 (skipped by
        # the SWDGE ucode, saving descriptors + bytes).  num_idxs per class
        # is the STATIC ncmax so the skip tail is identical on all cores.
        gparts = []
        rix = np.full(NPATCH, zero_row, np.int64)  # c==0 -> zero row

        def place(cid, j, pch):
            # class entry j lives at acc column aq+j//P partition j%P; the
            # per-class stage write walks partition-major: row =
            # base + part*Qc + q
            rix[pch] = aq[cid] * P + (j % P) * Q[cid] + j // P

        # c78 block first: c8 gather (dummy prefix over [0,n7max) then c8
        # windows, -1 tail), then c7 gather overwrites [0, n7max)
        slots = P * Q[78]
        n8slots = max(1, n7max + n8max)
        g8 = np.full(slots, -1, np.int64)
        g8[:n8slots] = DUMMY
        for j, pch in enumerate(r["cls"][8]):
            g8[n7max + j] = st[pch]
            place(78, n7max + j, pch)
        g7 = np.full(max(1, n7max), DUMMY, np.int64)
        for j, pch in enumerate(r["cls"][7]):
            g7[j] = st[pch]
            place(78, j, pch)
        gparts.append(wrap16(g8, slots))
        gparts.append(wrap16(g7, max(1, n7max)))
        for c in (6, 5, 4, 3, 2):
            slots = P * Q[c]
            gi = np.full(slots, -1, np.int64)
            gi[:max(1, ncls[c])] = DUMMY
            for j, pch in enumerate(r["cls"][c]):
                gi[j] = st[pch]
                place(c, j, pch)
            gparts.append(wrap16(gi, slots))
        for j, pch in enumerate(r["ovr"]):
            rix[pch] = ovr_base + j

        # regather idx: 4 regions of 256 patches each
        rparts = [wrap16(rix[k * 256:(k + 1) * 256], 256) for k in range(4)]

        # host-prefilled staging: override rows + zero row (class regions
        # are garbage until the device stages them)
        stg = np.zeros((stg_rows, DIM), np.float32)
        stg[ovr_base:ovr_base + len(r["ovr"])] = _reference_rows(
            h[b], st, cn, r["ovr"])

        in_maps.append(dict(
            hb=hb,
            gidx=np.ascontiguousarray(np.concatenate(gparts, 1)),
            rix=np.ascontiguousarray(np.concatenate(rparts, 1)),
            stg=stg,
        ))
    return in_maps, sizes


# ---------------------------------------------------------------- device IR

class ClassTile:
    """fp16 gather tile [P, Q, W, DIM] + plane AP helpers (rank<=3)."""

    def __init__(self, pool, name, Q, W, dt):
        self.Q, self.W = Q, W
        self.t = pool.tile([P, Q, W, DIM], dt, tag=name)

    def planes(self, w, width=1):
        a = self.t[:]
        return bass.AP(a.tensor, a.offset + w * DIM,
                       [a.ap[0], [self.W * DIM, self.Q], [1, width * DIM]])

    def all(self):
        """[P, Q, W*DIM] view (dma_gather dst contract)."""
        a = self.t[:]
        return bass.AP(a.tensor, a.offset,
                       [a.ap[0], [self.W * DIM, self.Q], [1, self.W * DIM]])


class Scratch:
    """fp16 scratch planes [P, NS, DIM] shaped as Q-blocks on demand."""

    def __init__(self, pool, name, nplanes, dt):
        self.n = nplanes
        self.t = pool.tile([P, nplanes, DIM], dt, tag=name)

    def planes(self, s, Q, W, width=1):
        """View scratch planes starting at s as a [P, Q, width*DIM] AP whose
        q-stride is W*DIM (matching a ClassTile's q layout)."""
        a = self.t[:]
        return bass.AP(a.tensor, a.offset + s * DIM,
                       [a.ap[0], [W * DIM, Q], [1, width * DIM]])


def build_kernel(ctx, tc, out_ap, in_aps, sizes):
    nc = tc.nc
    dt = mybir.dt
    Q, n7max, n8max, QO = sizes["Q"], sizes["n7max"], sizes["n8max"], sizes["QO"]
    ncls, SQ = sizes["ncls"], sizes["SQ"]

    pool = ctx.enter_context(tc.tile_pool(name="main", bufs=1))

    # load the SWDGE ISA ucode library up front: the Q7 overlay load takes
    # ~9us and would otherwise be auto-inserted right before the first
    # dma_gather, serializing with the idx-table loads
    nc.gpsimd.load_library(mlp)

    # ---- tables
    gcols = sum(8 * Q[c] for c in (2, 3, 4, 5, 6)) + 8 * Q[78] \
        + (max(1, n7max) + 15) // 16
    gidx = pool.tile([P, gcols], dt.int16, tag="gidx")
    rix = pool.tile([P, 64], dt.int16, tag="rix")
    nc.sync.dma_start(gidx[:], in_aps["gidx"][:])
    nc.sync.dma_start(rix[:], in_aps["rix"][:])

    # fp32 results for the device classes, column order (c78,c6,c5,c4,c3,c2)
    acc = pool.tile([P, SQ, DIM], dt.float32, tag="acc")
    aq = {}
    q0 = 0
    for cid in (78, 6, 5, 4, 3, 2):
        aq[cid] = q0
        q0 += Q[cid]

    # ---- gather tiles
    f16 = dt.float16
    tiles = {c: ClassTile(pool, f"x{c}", Q[c], c, f16) for c in (2, 3, 4, 5, 6)}
    t78 = ClassTile(pool, "x78", Q[78], 8, f16)
    # scratch sized for the widest strided use: class c5 (Q=2, W=5) uses
    # planes up to (Q-1)*5 + 4; keep headroom for Q6/Q78 = 2 variants
    scr = Scratch(pool, "scr", 16, f16)
    rf16 = pool.tile([P, SQ, DIM], f16, tag="rf16")

    def hbw(c):
        """Windowed view of hb: rows of c*DIM at stride DIM."""
        a = in_aps["hb"][:]
        return bass.AP(a.tensor, 0, [[DIM, HB_ROWS - (c - 1)], [1, c * DIM]])

    go = 0

    def gather(c, dst_ap, n_idx, cols, queue):
        nonlocal go
        idxs = gidx[:, go:go + (n_idx + 15) // 16]
        go += cols
        return nc.gpsimd.dma_gather(dst_ap, hbw(c), idxs, n_idx, n_idx,
                                    c * DIM, elem_step=DIM, queue_num=queue)

    # class gathers spread over 4 SWDGE queues (balanced by bytes); c7
    # overwrites the c8 dummy prefix, so it must wait for the c8 DMA
    # (same-queue FIFO is not write-ordered across engines) -- explicit sem.
    c8sem = nc.alloc_semaphore("c8done")
    c7sem = nc.alloc_semaphore("c7done")
    gather(8, t78.all(), max(1, n7max + n8max), 8 * Q[78], 1).then_inc(c8sem, 16)
    g7cols = (max(1, n7max) + 15) // 16
    g7go = go
    go += g7cols
    gather(6, tiles[6].all(), max(1, ncls[6]), 8 * Q[6], 0)
    gather(5, tiles[5].all(), max(1, ncls[5]), 8 * Q[5], 2)
    gather(4, tiles[4].all(), max(1, ncls[4]), 8 * Q[4], 3)
    gather(3, tiles[3].all(), max(1, ncls[3]), 8 * Q[3], 0)
    gather(2, tiles[2].all(), max(1, ncls[2]), 8 * Q[2], 2)
    if n7max > 0:
        nc.gpsimd.wait_ge(c8sem, 16)
        nc.gpsimd.dma_gather(
            bass.AP(t78.all().tensor, t78.all().offset,
                    [t78.all().ap[0], [8 * DIM * Q[78], 1], [1, 7 * DIM]]),
            hbw(7), gidx[:, g7go:g7go + (n7max + 15) // 16],
            n7max, n7max, 7 * DIM, elem_step=DIM,
            queue_num=1).then_inc(c7sem, 16)

    # c7 entries: plane 7 := -FLT16_MAX.  Explicit sem: tile does not order
    # an engine write against a later-completing ISA-DMA write (WAW), and
    # both c8/c7 gathers write this plane.
    if n7max > 0:
        nc.vector.wait_ge(c7sem, 16)
        a = t78.all()
        p7 = bass.AP(a.tensor, a.offset + 7 * DIM,
                     [[a.ap[0][0], n7max], [1, DIM]])
        nc.vector.memset(p7, NEGF16)

    TT = mybir.AluOpType

    def tt(dst, a, b, op, eng=None):
        (eng or nc.vector).tensor_tensor(dst, a, b, op=op)

    # ---------- W8 network (c7 padded + c8), Q=Q[78]
    def w8_net(x: ClassTile, dst):
        Qx, W = x.Q, x.W
        s = lambda i, width=1: scr.planes(i, Qx, 6, width)
        # sort4 asc on planes 0-3 (a0<=a1<=a2<=a3), comparators
        # (0,2),(1,3),(0,1),(2,3),(1,2); desc on 4-7 mirrored.
        tt(s(0, 2), x.planes(0, 2), x.planes(2, 2), TT.min)     # s01=min(01,23)
        tt(x.planes(2, 2), x.planes(0, 2), x.planes(2, 2), TT.max)
        tt(x.planes(0), s(0), s(1), TT.min)                     # a0
        tt(x.planes(1), s(0), s(1), TT.max)
        tt(s(0), x.planes(2), x.planes(3), TT.min)
        tt(x.planes(3), x.planes(2), x.planes(3), TT.max)       # a3
        tt(s(1), x.planes(1), s(0), TT.min)                     # a1
        tt(x.planes(2), x.planes(1), s(0), TT.max)              # a2
        # now asc: a0=x0, a1=s1, a2=x2, a3=x3
        tt(s(2, 2), x.planes(4, 2), x.planes(6, 2), TT.max)     # s23=max(45,67)
        tt(x.planes(6, 2), x.planes(4, 2), x.planes(6, 2), TT.min)
        tt(x.planes(4), s(2), s(3), TT.max)                     # d0
        tt(x.planes(5), s(2), s(3), TT.min)
        tt(s(2), x.planes(6), x.planes(7), TT.max)
        tt(x.planes(7), x.planes(6), x.planes(7), TT.min)       # d3
        tt(s(3), x.planes(5), s(2), TT.max)                     # d1
        tt(x.planes(6), x.planes(5), s(2), TT.min)              # d2
        # desc: d0=x4, d1=s3, d2=x6, d3=x7
        # H_i = max(a_i, d_i)
        tt(s(4), x.planes(0), x.planes(4), TT.max)              # H0
        tt(s(5), s(1), s(3), TT.max)                            # H1
        tt(x.planes(0), x.planes(2), x.planes(6), TT.max)       # H2
        tt(x.planes(1), x.planes(3), x.planes(7), TT.max)       # H3
        tt(s(4), s(4), s(5), TT.add)
        tt(x.planes(0), x.planes(0), x.planes(1), TT.add)
        tt(dst, s(4), x.planes(0), TT.add)

    # ---------- c6: sort4 asc (0-3) + sort2 desc (4,5); top4={max(a0,b0),
    # max(a1,b1), a2, a3}
    def c6_net(x: ClassTile, dst):
        Qx = x.Q
        s = lambda i, width=1: scr.planes(i, Qx, 6, width)
        tt(s(0, 2), x.planes(0, 2), x.planes(2, 2), TT.min)
        tt(x.planes(2, 2), x.planes(0, 2), x.planes(2, 2), TT.max)
        tt(x.planes(0), s(0), s(1), TT.min)
        tt(x.planes(1), s(0), s(1), TT.max)
        tt(s(0), x.planes(2), x.planes(3), TT.min)
        tt(x.planes(3), x.planes(2), x.planes(3), TT.max)
        tt(s(1), x.planes(1), s(0), TT.min)                     # a1
        tt(x.planes(2), x.planes(1), s(0), TT.max)              # a2
        # sort2 desc on (4,5): b0=max, b1=min
        tt(s(2), x.planes(4), x.planes(5), TT.max)              # b0
        tt(s(3), x.planes(4), x.planes(5), TT.min)              # b1
        tt(s(4), x.planes(0), s(2), TT.max)                     # H0=max(a0,b0)
        tt(s(5), s(1), s(3), TT.max)                            # H1=max(a1,b1)
        tt(s(4), s(4), s(5), TT.add)
        tt(s(4), s(4), x.planes(2), TT.add)
        tt(dst, s(4), x.planes(3), TT.add)

    # ---------- c5: (sum5 - min5)
    def c5_net(x: ClassTile, dst):
        Qx = x.Q
        s = lambda i, width=1: scr.planes(i, Qx, 5, width)
        tt(s(0, 2), x.planes(0, 2), x.planes(2, 2), TT.add)
        tt(s(0), s(0), s(1), TT.add)
        tt(s(0), s(0), x.planes(4), TT.add)                     # sum5
        tt(s(2, 2), x.planes(0, 2), x.planes(2, 2), TT.min)
        tt(s(2), s(2), s(3), TT.min)
        tt(s(2), s(2), x.planes(4), TT.min)                     # min5
        tt(dst, s(0), s(2), TT.subtract)

    # small sum classes run on the GpSimd engine (frees the DVE for the
    # sort networks); scratch planes 8+9 avoid the DVE classes' scratch
    def c4_net(x: ClassTile, dst):
        Qx = x.Q
        s = lambda i, width=1: scr.planes(10 + i, Qx, 4, width)
        tt(s(0, 2), x.planes(0, 2), x.planes(2, 2), TT.add, eng=nc.gpsimd)
        tt(dst, s(0), s(1), TT.add, eng=nc.gpsimd)

    def c3_net(x: ClassTile, dst):
        tt(dst, x.planes(0), x.planes(1), TT.add, eng=nc.gpsimd)
        tt(dst, dst, x.planes(2), TT.add, eng=nc.gpsimd)

    def c2_net(x: ClassTile, dst):
        tt(dst, x.planes(0), x.planes(1), TT.add, eng=nc.gpsimd)

    def rview(t, q0, Qc):
        a = t[:]
        return bass.AP(a.tensor, a.offset + q0 * DIM,
                       [a.ap[0], [DIM, Qc], [1, DIM]])

    # run networks (gather-arrival order); after each class: scalar
    # scale+cast to fp32 acc, then stage the class's rows to staging DRAM
    # (direct DMA on the scalar HWDGE queue, row = base + part*Qc + q).
    # gpsimd classes (c4/c3/c2) are emitted after the DVE ones so their
    # instructions queue behind the gathers on the gpsimd engine.
    stg = in_aps["stg"]
    order = [(6, tiles[6], c6_net, 0.25), (5, tiles[5], c5_net, 0.25),
             (78, t78, w8_net, 0.25), (4, tiles[4], c4_net, 0.25),
             (3, tiles[3], c3_net, 1.0 / 3.0), (2, tiles[2], c2_net, 0.5)]
    for cid, xt, net, scale in order:
        Qc = xt.Q
        r16 = rview(rf16, aq[cid], Qc)
        net(xt, r16)
        a32 = rview(acc, aq[cid], Qc)
        nc.scalar.mul(a32, r16, scale)
        dst = bass.AP(stg.tensor, aq[cid] * P * DIM, [[DIM, P * Qc], [1, DIM]])
        nc.scalar.dma_start(dst, a32)

    # ---- regather staging rows into natural patch order (4 parallel
    # queues, descriptors prepped early + triggered after staging) and
    # write out with plain direct DMAs
    onat = pool.tile([P, 4, 2, DIM], dt.float32, tag="onat")
    rsem = [nc.alloc_semaphore(f"rg{k}") for k in range(4)]
    outs = []
    for k in range(4):
        dstk = bass.AP(onat[:].tensor, onat[:].offset + k * 2 * DIM,
                       [onat[:].ap[0], [DIM, 2], [1, DIM]])
        nc.gpsimd.dma_gather(dstk, stg[:], rix[:, k * 16:(k + 1) * 16],
                             256, 256, DIM, queue_num=k, prepare_only=True,
                             sem=rsem[k])
        outs.append(dstk)
    for k in range(4):
        nc.gpsimd.trigger_dma(count=1, queue_num=k)
    for k in range(4):
        eng = nc.sync if k % 2 == 0 else nc.scalar
        eng.wait_ge(rsem[k], 16)
        outk = bass.AP(out_ap.tensor, k * 256 * DIM,
                       [[DIM, P], [P * DIM, 2], [1, DIM]])
        eng.dma_start(outk, outs[k])


def build_module(sizes, num_devices=8):
    nc = bacc.Bacc("TRN2", num_devices=num_devices, debug=False,
                   enable_asserts=False, num_swdge_queues=4)
    dt = mybir.dt
    Q = sizes["Q"]
    gcols = sum(8 * Q[c] for c in (2, 3, 4, 5, 6)) + 8 * Q[78] \
        + (max(1, sizes["n7max"]) + 15) // 16
    in_aps = {}
    specs = dict(
        hb=((HB_ROWS, DIM), dt.float16),
        gidx=((P, gcols), dt.int16),
        rix=((P, 64), dt.int16),
        stg=((sizes["stg_rows"], DIM), dt.float32),
    )
    for name, (shape, dtype) in specs.items():
        in_aps[name] = nc.dram_tensor(name, list(shape), dtype,
                                      kind="ExternalInput").ap()
    out_ap = nc.dram_tensor("out", [NPATCH, DIM], dt.float32,
                            kind="ExternalOutput").ap()
    with tile.TileContext(nc) as tc:
        with ExitStack() as ctx:
            build_kernel(ctx, tc, out_ap, in_aps, sizes)
    nc.compile()
    return nc


def _enable_axon_profiling():
    """Register the NTFF profile hook (the container image lacks
    antenv.axon_hooks; recreate it and wire the ctypes hook)."""
    import sys
    import types

    import antenv

    if 'antenv.axon_hooks' not in sys.modules:
        mod = types.ModuleType('antenv.axon_hooks')
        mod._hook = None
        mod.set_axon_ntff_profile_hook = lambda h: setattr(mod, '_hook', h)
        mod.get_axon_ntff_profile_hook = lambda: mod._hook
        sys.modules['antenv.axon_hooks'] = mod
        antenv.axon_hooks = mod
    from antenv import axon_hooks
    if axon_hooks.get_axon_ntff_profile_hook() is None:
        from trn_agent_boot.trn_boot import _ntff_profile_via_ctypes
        axon_hooks.set_axon_ntff_profile_hook(
            _ntff_profile_via_ctypes('/opt/axon/libaxon_pjrt.so'))
    import concourse.bass_utils as bu
    bu.upload_artifacts = lambda tmpdir: tmpdir


def kernel(h, patch_ids, max_num_patches, k, _profile=False):
    assert int(np.asarray(k)) == K
    assert int(np.asarray(max_num_patches)) == NPATCH
    nb = np.asarray(h).shape[0]
    if _profile:
        try:
            _enable_axon_profiling()
        except Exception as e:
            print(f"profiling setup failed ({e}); running without trace")
            _profile = False
    in_maps, sizes = prepare(h, patch_ids)
    nc = build_module(sizes, num_devices=nb)
    res = run_bass_kernel_spmd(nc, in_maps, core_ids=list(range(nb)),
                               trace=_profile)
    out = np.stack([res.results[b]["out"][:NPATCH] for b in range(nb)], 0)
    if _profile:
        kernel.last_results = res
    return out.astype(np.float32)


# revision 34
# speedup vs baseline: 1.3008x; 1.3008x over previous
"""Trainium2 Bass kernel for ByteLatentEncoder topk_mean_pooling (segment top-4 mean).

Problem: h [8, 4096, 512] f32, patch_ids [8, 4096] int64 (sorted per row,
values in [0, 1024)).  Output [8, 1024, 512]: per (batch, patch, channel),
mean of the top-min(4, count) *distinct* segment values with the reference's
knockout semantics (ties collapse; exhausted ranks contribute exactly -1e9).

v2 design (one NeuronCore per batch row):
  - Patches are grouped by EXACT count c into device classes c=2..8.  Each
    class gathers its segments as contiguous c-row windows from an fp16 copy
    of h with ONE dma_gather (SWDGE ISA ucode) per class -- no masks, no
    per-token column DMAs, half the bytes of fp32.
  - Tie-free top-4 means are order statistics, so they are computed with
    fp16 partial-sort (bitonic) networks on stock tensor_tensor ops, which
    run at 2 elem/cycle on the DVE (fp32 exactness is only needed for the
    reference's tie-knockout cases, which are routed to the host path).
      c=2..4: out = (sum of all c)/c          (plain adds)
      c=5:    out = (sum5 - min5)/4           (adds + min tree)
      c=6:    top4 = bitonic split of sort4(asc)++sort2(desc) padded
      c=7,8:  one shared W=8 block: sort4(asc) ++ sort4(desc), H=max split;
              c=7 windows read one foreign row that is overwritten by a
              -FLT16_MAX memset plane before the network runs.
  - Host precomputes (exact fp32 reference replica) the rare rows the fp16
    path can't represent: c=1 (copy), c>=9 (top-4 of a wide segment), and
    any patch with an exact in-segment duplicate (knockout -1e9 semantics).
    c=0 rows are zero -- covered by the output zero-init.
  - Results are scaled+cast to fp32 on the Scalar engine (1/min(4,c) per
    class) and written with one dma_scatter_add per class onto the
    zero-initialized output (add-to-zero == copy; pad slots land in a
    sacrificial 1025th row).
"""

import math
from contextlib import ExitStack

import numpy as np

import concourse.bacc as bacc
import concourse.bass as bass
import concourse.mybir as mybir
import concourse.tile as tile
from concourse.bass_utils import run_bass_kernel_spmd
from concourse.library_config import mlp

P = 128
SEQ = 4096
DIM = 512
NPATCH = 1024
K = 4
NEG = -1.0e9
NEGF16 = -65504.0
HB_ROWS = SEQ + 8  # 8 pad rows so full-8 windows of the last patch stay in range
DUMMY = SEQ  # dummy gather row (zeros pad region)
TRASH = NPATCH  # sacrificial scatter row

DEV_CLASSES = (2, 3, 4, 5, 6, 7, 8)


# ---------------------------------------------------------------- host side

def _reference_rows(h_row, starts, counts, pids):
    """Exact fp32 replica of reference() for the given patch ids."""
    out = np.zeros((len(pids), DIM), np.float32)
    for i, p in enumerate(pids):
        c = int(counts[p])
        if c == 0:
            continue
        seg = h_row[starts[p]:starts[p] + c].astype(np.float32)
        work = seg.copy()
        acc = np.zeros(DIM, np.float32)
        n = min(K, c)
        for r in range(n):
            cm = work.max(axis=0)
            acc += cm
            work = np.where(work == cm[None, :], np.float32(NEG), work)
        out[i] = acc / np.float32(n)
    return out


def _find_tie_patches(h_row, starts, counts, cand):
    """Among candidate patch ids (2<=c<=8), those with an exact per-channel
    duplicate anywhere in the segment (conservative superset of the patches
    where reference knockout != plain top-4)."""
    ties = []
    for c in range(2, 9):
        sel = cand[counts[cand] == c]
        if len(sel) == 0:
            continue
        idx = starts[sel, None] + np.arange(c)[None, :]
        seg = h_row[idx]  # [n, c, DIM]
        s = np.sort(seg, axis=1)
        dup = (s[:, 1:, :] == s[:, :-1, :]).any(axis=(1, 2))
        ties.extend(sel[dup].tolist())
    return ties


def build_row(h_row, pid_row):
    starts = np.searchsorted(pid_row, np.arange(NPATCH + 1)).astype(np.int64)
    counts = np.diff(starts)
    starts = starts[:-1]
    cand = np.where((counts >= 2) & (counts <= 8))[0]
    ties = set(_find_tie_patches(h_row, starts, counts, cand))
    cls = {c: [] for c in DEV_CLASSES}
    ovr = []
    for p in range(NPATCH):
        c = int(counts[p])
        if c == 0:
            continue
        if c == 1 or c >= 9 or p in ties:
            ovr.append(p)
        else:
            cls[c].append(p)
    return dict(starts=starts, counts=counts, cls=cls, ovr=ovr)


def wrap16(idx, n_slots):
    """SWDGE idx layout: slot j at [j%16, j//16], replicated to 8 stripes."""
    cols = (n_slots + 15) // 16
    t = np.zeros((16, cols), np.int16)
    for j, v in enumerate(idx):
        t[j % 16, j // 16] = v
    return np.tile(t, (8, 1))


def prepare(h, patch_ids):
    h = np.ascontiguousarray(np.asarray(h, np.float32))
    pid = np.asarray(patch_ids)
    nb = h.shape[0]
    rows = [build_row(h[b], pid[b]) for b in range(nb)]

    # global (compile-time) sizes
    ncls = {c: max(len(r["cls"][c]) for r in rows) for c in DEV_CLASSES}
    Q = {c: max(1, math.ceil(ncls[c] / P)) for c in (2, 3, 4, 5, 6)}
    n7max, n8max = ncls[7], ncls[8]
    assert n7max <= P and n8max <= P, (n7max, n8max)
    assert n7max + n8max <= P, "c7+c8 exceed one q-block; add Q78 support"
    Q[78] = 1
    novr = max(1, max(len(r["ovr"]) for r in rows))
    QO = math.ceil(novr / P)
    sizes = dict(Q=Q, n7max=n7max, n8max=n8max, QO=QO)

    in_maps = []
    for b, r in enumerate(rows):
        st, cn = r["starts"], r["counts"]
        hb = np.concatenate(
            [h[b], np.zeros((HB_ROWS - SEQ, DIM), np.float32)], 0
        ).astype(np.float16)

    # staging row map: class regions (order 78,6,5,4,3,2), override region,
    # one zero row; rix[p] = staging row whose content is out[p]
    aq = {}
    q0 = 0
    for cid in (78, 6, 5, 4, 3, 2):
        aq[cid] = q0
        q0 += Q[cid]
    SQ = q0
    ovr_base = SQ * P
    zero_row = ovr_base + QO * P
    stg_rows = zero_row + 1
    gcols = sum(8 * Q[c] for c in (2, 3, 4, 5, 6)) \
        + (max(1, n8max) + 15) // 16 + (max(1, n7max) + 15) // 16
    sizes.update(ncls=ncls, SQ=SQ, stg_rows=stg_rows, gcols=gcols)

    for b, r in enumerate(rows):
        st, cn = rows[b]["starts"], rows[b]["counts"]
        # gather idx tables: reals first, per-row shortfall [n_c, ncmax)
        # reads the dummy row, static tail [ncmax, 128*Q) is -1 (skipped by
        # the SWDGE ucode, saving descriptors + bytes).  num_idxs per class
        # is the STATIC ncmax so the skip tail is identical on all cores.
        gparts = []
        rix = np.full(NPATCH, zero_row, np.int64)  # c==0 -> zero row

        def place(cid, j, pch):
            # class entry j lives at acc column aq+j//P partition j%P; the
            # per-class stage write walks partition-major: row =
            # base + part*Qc + q
            rix[pch] = aq[cid] * P + (j % P) * Q[cid] + j // P

        # c78 block: c7 windows (7 rows, planes 0-6 of slots [0, n7max));
        # c8 gathers into its own tile and is SBUF-copied into slots
        # [n7max, n7max+n8max) -- no write overlap between the two DMAs
        slots = P * Q[78]
        g8 = np.full(max(1, n8max), DUMMY, np.int64)
        for j, pch in enumerate(r["cls"][8]):
            g8[j] = st[pch]
            place(78, n7max + j, pch)
        g7 = np.full(max(1, n7max), DUMMY, np.int64)
        for j, pch in enumerate(r["cls"][7]):
            g7[j] = st[pch]
            place(78, j, pch)
        gparts.append(wrap16(g8, max(1, n8max)))
        gparts.append(wrap16(g7, max(1, n7max)))
        for c in (6, 5, 4, 3, 2):
            slots = P * Q[c]
            gi = np.full(slots, -1, np.int64)
            gi[:max(1, ncls[c])] = DUMMY
            for j, pch in enumerate(r["cls"][c]):
                gi[j] = st[pch]
                place(c, j, pch)
            gparts.append(wrap16(gi, slots))
        for j, pch in enumerate(r["ovr"]):
            rix[pch] = ovr_base + j

        # regather idx: 4 regions of 256 patches each
        rparts = [wrap16(rix[k * 256:(k + 1) * 256], 256) for k in range(4)]

        # host-prefilled staging: override rows + zero row (class regions
        # are garbage until the device stages them)
        stg = np.zeros((stg_rows, DIM), np.float32)
        stg[ovr_base:ovr_base + len(r["ovr"])] = _reference_rows(
            h[b], st, cn, r["ovr"])

        in_maps.append(dict(
            hb=hb,
            gidx=np.ascontiguousarray(np.concatenate(gparts, 1)),
            rix=np.ascontiguousarray(np.concatenate(rparts, 1)),
            stg=stg,
        ))
    return in_maps, sizes


# ---------------------------------------------------------------- device IR

class ClassTile:
    """fp16 gather tile [P, Q, W, DIM] + plane AP helpers (rank<=3)."""

    def __init__(self, pool, name, Q, W, dt):
        self.Q, self.W = Q, W
        self.t = pool.tile([P, Q, W, DIM], dt, tag=name)

    def planes(self, w, width=1):
        a = self.t[:]
        return bass.AP(a.tensor, a.offset + w * DIM,
                       [a.ap[0], [self.W * DIM, self.Q], [1, width * DIM]])

    def all(self):
        """[P, Q, W*DIM] view (dma_gather dst contract)."""
        a = self.t[:]
        return bass.AP(a.tensor, a.offset,
                       [a.ap[0], [self.W * DIM, self.Q], [1, self.W * DIM]])


class Scratch:
    """fp16 scratch planes [P, NS, DIM] shaped as Q-blocks on demand."""

    def __init__(self, pool, name, nplanes, dt):
        self.n = nplanes
        self.t = pool.tile([P, nplanes, DIM], dt, tag=name)

    def planes(self, s, Q, W, width=1):
        """View scratch planes starting at s as a [P, Q, width*DIM] AP whose
        q-stride is W*DIM (matching a ClassTile's q layout)."""
        a = self.t[:]
        return bass.AP(a.tensor, a.offset + s * DIM,
                       [a.ap[0], [W * DIM, Q], [1, width * DIM]])


def build_kernel(ctx, tc, out_ap, in_aps, sizes):
    nc = tc.nc
    dt = mybir.dt
    Q, n7max, n8max, QO = sizes["Q"], sizes["n7max"], sizes["n8max"], sizes["QO"]
    ncls, SQ = sizes["ncls"], sizes["SQ"]

    pool = ctx.enter_context(tc.tile_pool(name="main", bufs=1))

    # load the SWDGE ISA ucode library up front: the Q7 overlay load takes
    # ~9us and would otherwise be auto-inserted right before the first
    # dma_gather, serializing with the idx-table loads
    nc.gpsimd.load_library(mlp)

    # ---- tables (tiny warmup DMA first: the first DMA on a queue pays a
    # multi-us completion latency; let a throwaway load absorb it)
    gcols = sizes["gcols"]
    gidx = pool.tile([P, gcols], dt.int16, tag="gidx")
    rix = pool.tile([P, 64], dt.int16, tag="rix")
    warm = pool.tile([P, 16], dt.int16, tag="warm")
    ga = in_aps["gidx"]
    nc.sync.dma_start(warm[:], bass.AP(ga.tensor, 0, [[gcols, P], [1, 16]]))
    nc.sync.dma_start(gidx[:], in_aps["gidx"][:])
    nc.sync.dma_start(rix[:], in_aps["rix"][:])

    # fp32 results for the device classes, column order (c78,c6,c5,c4,c3,c2)
    acc = pool.tile([P, SQ, DIM], dt.float32, tag="acc")
    aq = {}
    q0 = 0
    for cid in (78, 6, 5, 4, 3, 2):
        aq[cid] = q0
        q0 += Q[cid]

    # ---- gather tiles
    f16 = dt.float16
    tiles = {c: ClassTile(pool, f"x{c}", Q[c], c, f16) for c in (2, 3, 4, 5, 6)}
    t78 = ClassTile(pool, "x78", Q[78], 8, f16)
    t8 = ClassTile(pool, "x8", 1, 8, f16)
    # scratch sized for the widest strided use: class c5 (Q=2, W=5) uses
    # planes up to (Q-1)*5 + 4; keep headroom for Q6/Q78 = 2 variants
    scr = Scratch(pool, "scr", 16, f16)
    rf16 = pool.tile([P, SQ, DIM], f16, tag="rf16")

    def hbw(c):
        """Windowed view of hb: rows of c*DIM at stride DIM."""
        a = in_aps["hb"][:]
        return bass.AP(a.tensor, 0, [[DIM, HB_ROWS - (c - 1)], [1, c * DIM]])

    go = 0

    def gather(c, dst_ap, n_idx, cols, queue):
        nonlocal go
        idxs = gidx[:, go:go + (n_idx + 15) // 16]
        go += cols
        return nc.gpsimd.dma_gather(dst_ap, hbw(c), idxs, n_idx, n_idx,
                                    c * DIM, elem_step=DIM, queue_num=queue)

    # c7 entries' plane 7 := -FLT16_MAX.  No DMA ever writes this plane
    # (the c7 gather is 7 rows wide), so plain vector program order
    # sequences it before the W8 network.
    if n7max > 0:
        a = t78.all()
        p7 = bass.AP(a.tensor, a.offset + 7 * DIM,
                     [[a.ap[0][0], n7max], [1, DIM]])
        nc.vector.memset(p7, NEGF16)

    # class gathers spread over 4 SWDGE queues (balanced by bytes)
    gather(8, t8.all(), max(1, n8max), (max(1, n8max) + 15) // 16, 1)
    if n7max > 0:
        nc.gpsimd.dma_gather(
            bass.AP(t78.all().tensor, t78.all().offset,
                    [t78.all().ap[0], [8 * DIM * Q[78], 1], [1, 7 * DIM]]),
            hbw(7), gidx[:, go:go + (n7max + 15) // 16],
            n7max, n7max, 7 * DIM, elem_step=DIM, queue_num=1)
    go += (max(1, n7max) + 15) // 16
    gather(6, tiles[6].all(), max(1, ncls[6]), 8 * Q[6], 0)
    gather(5, tiles[5].all(), max(1, ncls[5]), 8 * Q[5], 2)
    gather(4, tiles[4].all(), max(1, ncls[4]), 8 * Q[4], 3)
    gather(3, tiles[3].all(), max(1, ncls[3]), 8 * Q[3], 3)
    gather(2, tiles[2].all(), max(1, ncls[2]), 8 * Q[2], 0)

    # copy the c8 windows into the shared block at partitions
    # [n7max, n7max+n8max): SBUF->SBUF direct DMA (a DVE copy can't start
    # at an unaligned partition, and this keeps the copy off the DVE)
    if n8max > 0:
        a8, a78 = t8.all(), t78.all()
        src = bass.AP(a8.tensor, a8.offset, [[a8.ap[0][0], n8max], [1, 8 * DIM]])
        dst = bass.AP(a78.tensor, a78.offset + n7max * a78.ap[0][0],
                      [[a78.ap[0][0], n8max], [1, 8 * DIM]])
        nc.sync.dma_start(dst, src)

    TT = mybir.AluOpType

    def tt(dst, a, b, op, eng=None):
        (eng or nc.vector).tensor_tensor(dst, a, b, op=op)

    # ---------- W8 network (c7 padded + c8), Q=Q[78]
    def w8_net(x: ClassTile, dst):
        Qx, W = x.Q, x.W
        s = lambda i, width=1: scr.planes(i, Qx, 6, width)
        # sort4 asc on planes 0-3 (a0<=a1<=a2<=a3), comparators
        # (0,2),(1,3),(0,1),(2,3),(1,2); desc on 4-7 mirrored.
        tt(s(0, 2), x.planes(0, 2), x.planes(2, 2), TT.min)     # s01=min(01,23)
        tt(x.planes(2, 2), x.planes(0, 2), x.planes(2, 2), TT.max)
        tt(x.planes(0), s(0), s(1), TT.min)                     # a0
        tt(x.planes(1), s(0), s(1), TT.max)
        tt(s(0), x.planes(2), x.planes(3), TT.min)
        tt(x.planes(3), x.planes(2), x.planes(3), TT.max)       # a3
        tt(s(1), x.planes(1), s(0), TT.min)                     # a1
        tt(x.planes(2), x.planes(1), s(0), TT.max)              # a2
        # now asc: a0=x0, a1=s1, a2=x2, a3=x3
        tt(s(2, 2), x.planes(4, 2), x.planes(6, 2), TT.max)     # s23=max(45,67)
        tt(x.planes(6, 2), x.planes(4, 2), x.planes(6, 2), TT.min)
        tt(x.planes(4), s(2), s(3), TT.max)                     # d0
        tt(x.planes(5), s(2), s(3), TT.min)
        tt(s(2), x.planes(6), x.planes(7), TT.max)
        tt(x.planes(7), x.planes(6), x.planes(7), TT.min)       # d3
        tt(s(3), x.planes(5), s(2), TT.max)                     # d1
        tt(x.planes(6), x.planes(5), s(2), TT.min)              # d2
        # desc: d0=x4, d1=s3, d2=x6, d3=x7
        # H_i = max(a_i, d_i)
        tt(s(4), x.planes(0), x.planes(4), TT.max)              # H0
        tt(s(5), s(1), s(3), TT.max)                            # H1
        tt(x.planes(0), x.planes(2), x.planes(6), TT.max)       # H2
        tt(x.planes(1), x.planes(3), x.planes(7), TT.max)       # H3
        tt(s(4), s(4), s(5), TT.add)
        tt(x.planes(0), x.planes(0), x.planes(1), TT.add)
        tt(dst, s(4), x.planes(0), TT.add)

    # ---------- c6: sort4 asc (0-3) + sort2 desc (4,5); top4={max(a0,b0),
    # max(a1,b1), a2, a3}
    def c6_net(x: ClassTile, dst):
        Qx = x.Q
        s = lambda i, width=1: scr.planes(i, Qx, 6, width)
        tt(s(0, 2), x.planes(0, 2), x.planes(2, 2), TT.min)
        tt(x.planes(2, 2), x.planes(0, 2), x.planes(2, 2), TT.max)
        tt(x.planes(0), s(0), s(1), TT.min)
        tt(x.planes(1), s(0), s(1), TT.max)
        tt(s(0), x.planes(2), x.planes(3), TT.min)
        tt(x.planes(3), x.planes(2), x.planes(3), TT.max)
        tt(s(1), x.planes(1), s(0), TT.min)                     # a1
        tt(x.planes(2), x.planes(1), s(0), TT.max)              # a2
        # sort2 desc on (4,5): b0=max, b1=min
        tt(s(2), x.planes(4), x.planes(5), TT.max)              # b0
        tt(s(3), x.planes(4), x.planes(5), TT.min)              # b1
        tt(s(4), x.planes(0), s(2), TT.max)                     # H0=max(a0,b0)
        tt(s(5), s(1), s(3), TT.max)                            # H1=max(a1,b1)
        tt(s(4), s(4), s(5), TT.add)
        tt(s(4), s(4), x.planes(2), TT.add)
        tt(dst, s(4), x.planes(3), TT.add)

    # ---------- c5: (sum5 - min5)
    def c5_net(x: ClassTile, dst):
        Qx = x.Q
        s = lambda i, width=1: scr.planes(i, Qx, 5, width)
        tt(s(0, 2), x.planes(0, 2), x.planes(2, 2), TT.add)
        tt(s(0), s(0), s(1), TT.add)
        tt(s(0), s(0), x.planes(4), TT.add)                     # sum5
        tt(s(2, 2), x.planes(0, 2), x.planes(2, 2), TT.min)
        tt(s(2), s(2), s(3), TT.min)
        tt(s(2), s(2), x.planes(4), TT.min)                     # min5
        tt(dst, s(0), s(2), TT.subtract)

    # NOTE: gpsimd tensor ops live in a different Q7 ucode library than the
    # ISA DMAs -- mixing them makes Bacc thrash library swaps (~2-9us each).
    # Keep ALL elementwise work on the DVE.
    def c4_net(x: ClassTile, dst):
        Qx = x.Q
        s = lambda i, width=1: scr.planes(10 + i, Qx, 4, width)
        tt(s(0, 2), x.planes(0, 2), x.planes(2, 2), TT.add)
        tt(dst, s(0), s(1), TT.add)

    def c3_net(x: ClassTile, dst):
        tt(dst, x.planes(0), x.planes(1), TT.add)
        tt(dst, dst, x.planes(2), TT.add)

    def c2_net(x: ClassTile, dst):
        tt(dst, x.planes(0), x.planes(1), TT.add)

    def rview(t, q0, Qc):
        a = t[:]
        return bass.AP(a.tensor, a.offset + q0 * DIM,
                       [a.ap[0], [DIM, Qc], [1, DIM]])

    # run networks (gather-arrival order); after each class: scalar
    # scale+cast to fp32 acc, then stage the class's rows to staging DRAM
    # (direct DMA on the scalar HWDGE queue, row = base + part*Qc + q).
    # gpsimd classes (c4/c3/c2) are emitted after the DVE ones so their
    # instructions queue behind the gathers on the gpsimd engine.
    stg = in_aps["stg"]
    order = [(6, tiles[6], c6_net, 0.25), (5, tiles[5], c5_net, 0.25),
             (78, t78, w8_net, 0.25), (4, tiles[4], c4_net, 0.25),
             (3, tiles[3], c3_net, 1.0 / 3.0), (2, tiles[2], c2_net, 0.5)]
    for cid, xt, net, scale in order:
        Qc = xt.Q
        r16 = rview(rf16, aq[cid], Qc)
        net(xt, r16)
        a32 = rview(acc, aq[cid], Qc)
        nc.scalar.mul(a32, r16, scale)
        dst = bass.AP(stg.tensor, aq[cid] * P * DIM, [[DIM, P * Qc], [1, DIM]])
        nc.scalar.dma_start(dst, a32)

    # ---- regather staging rows into natural patch order (4 parallel
    # queues, descriptors prepped early + triggered after staging) and
    # write out with plain direct DMAs
    onat = pool.tile([P, 4, 2, DIM], dt.float32, tag="onat")
    rsem = [nc.alloc_semaphore(f"rg{k}") for k in range(4)]
    outs = []
    for k in range(4):
        dstk = bass.AP(onat[:].tensor, onat[:].offset + k * 2 * DIM,
                       [onat[:].ap[0], [DIM, 2], [1, DIM]])
        nc.gpsimd.dma_gather(dstk, stg[:], rix[:, k * 16:(k + 1) * 16],
                             256, 256, DIM, queue_num=k, prepare_only=True,
                             sem=rsem[k])
        outs.append(dstk)
    for k in range(4):
        nc.gpsimd.trigger_dma(count=1, queue_num=k)
    for k in range(4):
        eng = nc.sync if k % 2 == 0 else nc.scalar
        eng.wait_ge(rsem[k], 16)
        outk = bass.AP(out_ap.tensor, k * 256 * DIM,
                       [[DIM, P], [P * DIM, 2], [1, DIM]])
        eng.dma_start(outk, outs[k])


def build_module(sizes, num_devices=8):
    nc = bacc.Bacc("TRN2", num_devices=num_devices, debug=False,
                   enable_asserts=False, num_swdge_queues=4)
    dt = mybir.dt
    gcols = sizes["gcols"]
    in_aps = {}
    specs = dict(
        hb=((HB_ROWS, DIM), dt.float16),
        gidx=((P, gcols), dt.int16),
        rix=((P, 64), dt.int16),
        stg=((sizes["stg_rows"], DIM), dt.float32),
    )
    for name, (shape, dtype) in specs.items():
        in_aps[name] = nc.dram_tensor(name, list(shape), dtype,
                                      kind="ExternalInput").ap()
    out_ap = nc.dram_tensor("out", [NPATCH, DIM], dt.float32,
                            kind="ExternalOutput").ap()
    with tile.TileContext(nc) as tc:
        with ExitStack() as ctx:
            build_kernel(ctx, tc, out_ap, in_aps, sizes)
    nc.compile()
    return nc


def _enable_axon_profiling():
    """Register the NTFF profile hook (the container image lacks
    antenv.axon_hooks; recreate it and wire the ctypes hook)."""
    import sys
    import types

    import antenv

    if 'antenv.axon_hooks' not in sys.modules:
        mod = types.ModuleType('antenv.axon_hooks')
        mod._hook = None
        mod.set_axon_ntff_profile_hook = lambda h: setattr(mod, '_hook', h)
        mod.get_axon_ntff_profile_hook = lambda: mod._hook
        sys.modules['antenv.axon_hooks'] = mod
        antenv.axon_hooks = mod
    from antenv import axon_hooks
    if axon_hooks.get_axon_ntff_profile_hook() is None:
        from trn_agent_boot.trn_boot import _ntff_profile_via_ctypes
        axon_hooks.set_axon_ntff_profile_hook(
            _ntff_profile_via_ctypes('/opt/axon/libaxon_pjrt.so'))
    import concourse.bass_utils as bu
    bu.upload_artifacts = lambda tmpdir: tmpdir


def kernel(h, patch_ids, max_num_patches, k, _profile=False):
    assert int(np.asarray(k)) == K
    assert int(np.asarray(max_num_patches)) == NPATCH
    nb = np.asarray(h).shape[0]
    if _profile:
        try:
            _enable_axon_profiling()
        except Exception as e:
            print(f"profiling setup failed ({e}); running without trace")
            _profile = False
    in_maps, sizes = prepare(h, patch_ids)
    nc = build_module(sizes, num_devices=nb)
    res = run_bass_kernel_spmd(nc, in_maps, core_ids=list(range(nb)),
                               trace=_profile)
    out = np.stack([res.results[b]["out"][:NPATCH] for b in range(nb)], 0)
    if _profile:
        kernel.last_results = res
    return out.astype(np.float32)
